# revision 11
# baseline (speedup 1.0000x reference)
"""Chamfer distance (B=16, N=M=4096, D=3) on 8 Trainium2 NeuronCores.

Sharding: data-parallel over batch — 2 batches per core, SPMD (same NEFF,
different inputs per core).

Per batch, the NxM squared-distance matrix is produced by TensorE as a
single K=15 matmul using augmented embeddings with an fp16 hi/lo split:
    x~ = [x0,x1,x2, ||x||^2, 1],  y~ = [-2y0,-2y1,-2y2, 1, ||y||^2]
    A_n = [xh, xh, xl],  B_m = [yh, yl, yh]  (each 3x5 = K=15 rows)
    (A.B)[n,m] = xh.yh + xh.yl + xl.yh ~= x~.y~ = ||x_n - y_m||^2
(the dropped xl.yl term is ~5e-6; PSUM accumulates in fp32, so the
catastrophic cancellation of the expanded form stays at fp32 precision).

ScalarE casts PSUM->SBUF fp16 (relative precision is kept on the small
result values), VectorE computes the row-min per 128-row tile with a
pairwise tensor_tensor(min) halving tree (2x fp16 mode) plus one small
tensor_reduce, and an elementwise col-min accumulator across row tiles.
The tiny epilogue (partition-min of the col accumulator, sqrt, mean) runs
on host in fp32.
"""

import numpy as np

import concourse.mybir as mybir
import concourse.tile as tile
from concourse import bacc
from concourse.bass_utils import run_bass_kernel_spmd

B, N, M, D = 16, 4096, 4096, 3
N_CORES = 8
BPC = B // N_CORES  # batches per core
K = 15

F16 = mybir.dt.float16
F32 = mybir.dt.float32


def host_pack(x: np.ndarray, y: np.ndarray):
    """x, y: [B, N, 3] float32 -> A, B: [B, 15, N] float16 (lhsT/rhs layouts)."""
    xd = x.astype(np.float64)
    yd = y.astype(np.float64)
    ones_x = np.ones((*xd.shape[:2], 1))
    ones_y = np.ones((*yd.shape[:2], 1))
    xt = np.concatenate([xd, (xd * xd).sum(-1, keepdims=True), ones_x], axis=-1)
    yt = np.concatenate(
        [-2.0 * yd, ones_y, (yd * yd).sum(-1, keepdims=True)], axis=-1
    )
    xh = xt.astype(np.float16)
    xl = (xt - xh.astype(np.float64)).astype(np.float16)
    yh = yt.astype(np.float16)
    yl = (yt - yh.astype(np.float64)).astype(np.float16)
    A = np.concatenate([xh, xh, xl], axis=-1)  # [B, N, 15]
    Bm = np.concatenate([yh, yl, yh], axis=-1)
    return (
        np.ascontiguousarray(A.transpose(0, 2, 1)).astype(np.float16),
        np.ascontiguousarray(Bm.transpose(0, 2, 1)).astype(np.float16),
    )


def build_nc(bpc: int = BPC, n: int = N, m: int = M, k: int = K, reps: int = 1):
    NT = n // 128
    GW = 2048 if m % 2048 == 0 else m  # psum group width (4 banks)
    NG = m // GW
    MMW = 512  # matmul free width (one psum bank)

    nc = bacc.Bacc("TRN2", target_bir_lowering=False, debug=False)
    a_d = nc.dram_tensor("a", [bpc, k, n], F16, kind="ExternalInput")
    b_d = nc.dram_tensor("b", [bpc, k, m], F16, kind="ExternalInput")
    rm_d = nc.dram_tensor("rowmins", [bpc, 128, NT], F16, kind="ExternalOutput")
    cm_d = nc.dram_tensor("colmins", [bpc, 128, m], F16, kind="ExternalOutput")

    with tile.TileContext(nc) as tc:
        with (
            tc.tile_pool(name="ab", bufs=2) as ab_pool,
            tc.tile_pool(name="cast", bufs=3) as cast_pool,
            tc.tile_pool(name="acc", bufs=2) as acc_pool,
            tc.tile_pool(name="small", bufs=2) as small_pool,
            tc.tile_pool(name="scratch", bufs=1) as scratch_pool,
            tc.tile_pool(name="psum", bufs=2, space="PSUM") as psum_pool,
        ):
            for rep in range(reps):
              for bi in range(bpc):
                a_s = ab_pool.tile([k, n], F16, tag="a")
                b_s = ab_pool.tile([k, m], F16, tag="b")
                nc.sync.dma_start(a_s[:], a_d.ap()[bi])
                nc.sync.dma_start(b_s[:], b_d.ap()[bi])
                colacc = acc_pool.tile([128, m], F16)
                rowm = small_pool.tile([128, NT], F16)
                for nt in range(NT):
                    lhsT = a_s[:, nt * 128 : (nt + 1) * 128]
                    t16 = cast_pool.tile([128, m], F16, tag="t16")
                    for g in range(NG):
                        ps = psum_pool.tile([128, GW], F32, tag="ps")
                        for mb in range(GW // MMW):
                            m0 = mb * MMW
                            nc.tensor.matmul(
                                ps[:, m0 : m0 + MMW],
                                lhsT,
                                b_s[:, g * GW + m0 : g * GW + m0 + MMW],
                                start=True,
                                stop=True,
                            )
                        nc.scalar.copy(t16[:, g * GW : (g + 1) * GW], ps[:])
                    # col-min accumulate, one op across both groups
                    if nt == 0:
                        nc.vector.tensor_copy(colacc[:], t16[:])
                    else:
                        nc.vector.tensor_tensor(
                            colacc[:], t16[:], colacc[:], mybir.AluOpType.min
                        )
                    # row-min: pairwise halving tree at 2x, then a small reduce
                    u = scratch_pool.tile([128, m // 2], F16, tag="u", bufs=2)
                    w = m // 2
                    nc.vector.tensor_tensor(
                        u[:, :w], t16[:, :w], t16[:, w:], mybir.AluOpType.min
                    )
                    while w > 256:
                        h = w // 2
                        nc.vector.tensor_tensor(
                            u[:, :h], u[:, :h], u[:, h:w], mybir.AluOpType.min
                        )
                        w = h
                    nc.vector.tensor_reduce(
                        rowm[:, nt : nt + 1],
                        u[:, :w],
                        mybir.AxisListType.X,
                        mybir.AluOpType.min,
                    )
                nc.sync.dma_start(rm_d.ap()[bi], rowm[:])
                nc.sync.dma_start(cm_d.ap()[bi], colacc[:])
    nc.compile()
    return nc


def host_finish(rowmins: np.ndarray, colmins: np.ndarray):
    """rowmins [bpc,128,NT] f16, colmins [bpc,128,m] f16 -> cost [bpc] f32."""
    rm = np.clip(rowmins.astype(np.float32), 0.0, None)
    cm = np.clip(colmins.astype(np.float32).min(axis=1), 0.0, None)
    d1 = np.sqrt(rm.reshape(rm.shape[0], -1)).mean(axis=1)
    d2 = np.sqrt(cm).mean(axis=1)
    return ((d1 + d2) * 0.5).astype(np.float32)


_RUN_KWARGS = {}
_NC_CACHE = None


def _get_nc():
    global _NC_CACHE
    if _NC_CACHE is None:
        _NC_CACHE = build_nc()
    return _NC_CACHE


def kernel(x: np.ndarray, y: np.ndarray) -> np.ndarray:
    x = np.asarray(x, dtype=np.float32)
    y = np.asarray(y, dtype=np.float32)
    A, Bm = host_pack(x, y)
    nc = _get_nc()
    in_maps = [
        {"a": A[c * BPC : (c + 1) * BPC], "b": Bm[c * BPC : (c + 1) * BPC]}
        for c in range(N_CORES)
    ]
    res = run_bass_kernel_spmd(nc, in_maps, core_ids=list(range(N_CORES)), **_RUN_KWARGS)
    out = np.empty((B,), dtype=np.float32)
    for c in range(N_CORES):
        out[c * BPC : (c + 1) * BPC] = host_finish(
            res.results[c]["rowmins"], res.results[c]["colmins"]
        )
    return out
